# revision 1
# baseline (speedup 1.0000x reference)
"""GLOW coupling-flow (FrEIA-style) forward pass on 8 TRN2 NeuronCores.

Problem: B=8192, D=1024, C=512, H=512, L=8 coupling layers, each:
    xp = x[:, perm_k];  x1, x2 = xp[:, :512], xp[:, 512:]
    r2 = relu([x2, cond] @ w1 + b1) @ w2 + b2          (subnet 2)
    ls2 = 0.636*atan(r2[:, :512]);  y1 = exp(ls2)*x1 + r2[:, 512:]
    r1 = relu([y1, cond] @ w1 + b1) @ w2 + b2          (subnet 1)
    ls1 = 0.636*atan(r1[:, :512]);  y2 = exp(ls1)*x2 + r1[:, 512:]
    jac += sum(ls1 + ls2, axis=1);  x = [y1, y2]

Strategy:
- Pure data parallel: batch sharded 1024 rows/core, no collectives.
- Activations kept transposed (features on partitions, batch on free axis):
  matmuls are weight-stationary  out^T = W^T @ inp^T  with K on partitions.
- float32r matmuls (1 cyc/row at N>=256 vs 4 for fp32; rel err ~1.6e-4).
- Column permutation per layer via DRAM round-trip + indirect-DMA row
  gathers (one per 128 rows), hidden behind compute by splitting the batch
  into NCHUNK pipelined chunks.
- jac: partition-sums of atan tiles via ones-matmul accumulated in PSUM
  across the entire kernel; scaled by 0.636 once at the end.
- relu and the t-half (bias + add) run on DVE; atan/exp on ScalarE.
"""
import sys

sys.path.insert(0, "/opt/trn_rl_repo")
import numpy as np

import concourse.bass as bass
import concourse.bacc as bacc
import concourse.tile as tile
import concourse.mybir as mybir
from concourse.bass_utils import run_bass_kernel_spmd

F32 = mybir.dt.float32
F32R = mybir.dt.float32r
I32 = mybir.dt.int32
AF = mybir.ActivationFunctionType
ALU = mybir.AluOpType

B, D, C, H, L = 8192, 1024, 512, 512, 8
S = D // 2
CLAMP = 1.0
ATAN_SCALE = 0.636
NCORE = 8
BS = B // NCORE          # 1024 batch rows per core
NCHUNK = 2
BC = BS // NCHUNK        # batch columns per chunk
KT_X = D // 128          # 8 k-tiles over D
KT_S = S // 128          # 4 k-tiles over S/H
P = 128

_NC_CACHE = {}


def build():
    nc = bacc.Bacc("TRN2", target_bir_lowering=False)

    x_in = nc.dram_tensor("x", [D, BS], F32R, kind="ExternalInput")
    cond_in = nc.dram_tensor("cond", [C, BS], F32R, kind="ExternalInput")
    # per (layer, subnet): [128, 8*512 (w1: kt-major, m inner) + 4*1024 (w2)]
    wts = nc.dram_tensor("wts", [L, 2, P, 8192], F32R, kind="ExternalInput")
    # per (layer, subnet) 12 cols: b1 (4 mt) then b2 (8 mt)
    bias_in = nc.dram_tensor("bias", [P, L * 2 * 12], F32, kind="ExternalInput")
    gidx_in = nc.dram_tensor("gidx", [P, L * 8], I32, kind="ExternalInput")
    ones_in = nc.dram_tensor("ones", [P, 1], F32R, kind="ExternalInput")
    out_x = nc.dram_tensor("out", [D, BS], F32R, kind="ExternalOutput")
    out_j = nc.dram_tensor("jac", [1, BS], F32, kind="ExternalOutput")
    xb = [nc.dram_tensor(f"xbuf{i}", [D, BS], F32R) for i in range(2)]

    def dram_T(t, rows_lo, rows_hi, col_lo, col_hi):
        # view DRAM [rows, BS] slice as [128, kt, cols]
        return t[rows_lo:rows_hi, col_lo:col_hi].rearrange(
            "(kt p) n -> p kt n", p=P)

    with tile.TileContext(nc) as tc:
        with (
            tc.tile_pool(name="const", bufs=1) as cpool,
            tc.tile_pool(name="wt", bufs=2) as wpool,
            tc.tile_pool(name="act", bufs=2) as apool,
            tc.tile_pool(name="ab", bufs=2) as abpool,
            tc.tile_pool(name="ps", bufs=5, space="PSUM") as pspool,
            tc.tile_pool(name="psj", bufs=1, space="PSUM") as psjpool,
        ):
            # --- persistent loads ---
            ct = cpool.tile([P, KT_S, BS], F32R, tag="cond")
            nc.sync.dma_start(ct[:], dram_T(cond_in, 0, C, 0, BS))
            bsb = cpool.tile([P, L * 2 * 12], F32, tag="bias")
            nc.sync.dma_start(bsb[:], bias_in[:])
            gsb = cpool.tile([P, L * 8], I32, tag="gidx")
            nc.sync.dma_start(gsb[:], gidx_in[:])
            ones_t = cpool.tile([P, 1], F32R, tag="ones")
            nc.sync.dma_start(ones_t[:], ones_in[:])
            z0 = cpool.tile([P, 1], F32, tag="z0")
            nc.gpsimd.memset(z0[:], 0.0)

            jac_ps = psjpool.tile([1, BS], F32, tag="jac")
            jac_started = [False] * NCHUNK

            for k in range(L):
                src = x_in if k == 0 else xb[(k - 1) % 2]
                dst = out_x if k == L - 1 else xb[k % 2]

                # weight prefetch for both subnets of this layer
                w_t = []
                for s in range(2):
                    w = wpool.tile([P, 8192], F32R, tag="w")
                    nc.sync.dma_start(w[:], wts[k, s])
                    w_t.append(w)

                # gather a (x1) and b (x2) for each chunk
                a_t, b_t = [], []
                for c in range(NCHUNK):
                    at = abpool.tile([P, KT_S, BC], F32R, tag="a")
                    bt = abpool.tile([P, KT_S, BC], F32R, tag="b")
                    for j in range(KT_S):
                        nc.gpsimd.indirect_dma_start(
                            out=at[:, j, :], out_offset=None, in_=src[:],
                            in_offset=bass.IndirectOffsetOnAxis(
                                ap=gsb[:, k * 8 + j:k * 8 + j + 1], axis=0),
                            element_offset=c * BC)
                        nc.gpsimd.indirect_dma_start(
                            out=bt[:, j, :], out_offset=None, in_=src[:],
                            in_offset=bass.IndirectOffsetOnAxis(
                                ap=gsb[:, k * 8 + 4 + j:k * 8 + 4 + j + 1],
                                axis=0),
                            element_offset=c * BC)
                    a_t.append(at)
                    b_t.append(bt)

                # subnets: s=0 -> subnet2 (input x2=b, mult operand x1=a,
                # writes y1 into a); s=1 -> subnet1 (input y1=a, mult operand
                # x2=b, writes y2 into b)
                for s in range(2):
                    bofs = (k * 2 + s) * 12
                    for c in range(NCHUNK):
                        inp = b_t[c] if s == 0 else a_t[c]
                        xmul = a_t[c] if s == 0 else b_t[c]
                        cs, ce = c * BC, (c + 1) * BC

                        # mm1: h = relu(W1^T @ [inp, cond] + b1)
                        h = apool.tile([P, KT_S, BC], F32R, tag="h")
                        for mt in range(KT_S):
                            ph = pspool.tile([P, BC], F32, tag="ps")
                            for kt in range(KT_X):
                                if kt < KT_S:
                                    rhs = inp[:, kt, :]
                                else:
                                    rhs = ct[:, kt - KT_S, cs:ce]
                                lo = kt * 512 + mt * P
                                nc.tensor.matmul(
                                    ph[:], w_t[s][:, lo:lo + P], rhs,
                                    start=(kt == 0), stop=(kt == KT_X - 1))
                            nc.vector.tensor_scalar(
                                out=h[:, mt, :], in0=ph[:],
                                scalar1=bsb[:, bofs + mt:bofs + mt + 1],
                                scalar2=0.0, op0=ALU.add, op1=ALU.max)

                        # mm2: r = W2^T @ h + b2 ; s-half -> atan -> A,
                        # exp -> E ; t-half fused into y update
                        A = apool.tile([P, KT_S, BC], F32R, tag="A")
                        E = apool.tile([P, KT_S, BC], F32, tag="E")
                        tmp = apool.tile([P, KT_S, BC], F32, tag="tmp")
                        for mt in range(2 * KT_S):
                            pr = pspool.tile([P, BC], F32, tag="ps")
                            for kt in range(KT_S):
                                lo = 4096 + kt * 1024 + mt * P
                                nc.tensor.matmul(
                                    pr[:], w_t[s][:, lo:lo + P], h[:, kt, :],
                                    start=(kt == 0), stop=(kt == KT_S - 1))
                            if mt < KT_S:
                                nc.scalar.activation(
                                    A[:, mt, :], pr[:], AF.Arctan,
                                    bias=bsb[:, bofs + 4 + mt:bofs + 5 + mt],
                                    scale=1.0)
                                nc.tensor.matmul(
                                    jac_ps[:, cs:ce], ones_t[:], A[:, mt, :],
                                    start=not jac_started[c], stop=False,
                                    skip_group_check=True)
                                jac_started[c] = True
                            else:
                                j = mt - KT_S
                                if j == 0:
                                    nc.scalar.activation(
                                        E[:], A[:], AF.Exp,
                                        bias=z0[:, :1], scale=ATAN_SCALE)
                                nc.vector.tensor_mul(
                                    tmp[:, j, :], E[:, j, :], xmul[:, j, :])
                                nc.vector.scalar_tensor_tensor(
                                    out=xmul[:, j, :], in0=pr[:],
                                    scalar=bsb[:, bofs + 4 + mt:bofs + 5 + mt],
                                    in1=tmp[:, j, :],
                                    op0=ALU.add, op1=ALU.add)

                # write y1 (in a) and y2 (in b) to dst
                for c in range(NCHUNK):
                    cs, ce = c * BC, (c + 1) * BC
                    nc.sync.dma_start(dram_T(dst, 0, S, cs, ce), a_t[c][:])
                    nc.sync.dma_start(dram_T(dst, S, D, cs, ce), b_t[c][:])

            # close the jac accumulation groups and emit 0.636 * sum
            jac_sb = cpool.tile([1, BS], F32, tag="jacsb")
            nc.scalar.mul(jac_sb[:], jac_ps[:], ATAN_SCALE)
            nc.sync.dma_start(out_j[:], jac_sb[:])

    nc.compile()
    return nc


def get_nc():
    if "nc" not in _NC_CACHE:
        _NC_CACHE["nc"] = build()
    return _NC_CACHE["nc"]


def _pack_weights(s1_w1, s1_b1, s1_w2, s1_b2, s2_w1, s2_b1, s2_w2, s2_b2):
    wts = np.empty((L, 2, P, 8192), dtype=np.float32)
    bias = np.empty((P, L * 2 * 12), dtype=np.float32)
    # s index 0 -> subnet2 (runs first), 1 -> subnet1
    for k in range(L):
        for s, (w1, b1, w2, b2) in enumerate(
            ((s2_w1, s2_b1, s2_w2, s2_b2), (s1_w1, s1_b1, s1_w2, s1_b2))
        ):
            # w1[k]: (1024, 512) -> [128, kt=8, m=512] -> flat [128, 4096]
            wts[k, s, :, :4096] = (
                w1[k].reshape(8, P, 512).transpose(1, 0, 2).reshape(P, 4096))
            # w2[k]: (512, 1024) -> [128, kt=4, m=1024] -> flat [128, 4096]
            wts[k, s, :, 4096:] = (
                w2[k].reshape(4, P, 1024).transpose(1, 0, 2).reshape(P, 4096))
            bofs = (k * 2 + s) * 12
            bias[:, bofs:bofs + 4] = b1[k].reshape(4, P).T
            bias[:, bofs + 4:bofs + 12] = b2[k].reshape(8, P).T
    return wts, bias


def _run(inputs, trace=False):
    x = np.asarray(inputs["x"], dtype=np.float32)
    cond = np.asarray(inputs["cond"], dtype=np.float32)
    perms = np.asarray(inputs["perms"], dtype=np.int32)
    wts, bias = _pack_weights(
        np.asarray(inputs["s1_w1"], np.float32), np.asarray(inputs["s1_b1"], np.float32),
        np.asarray(inputs["s1_w2"], np.float32), np.asarray(inputs["s1_b2"], np.float32),
        np.asarray(inputs["s2_w1"], np.float32), np.asarray(inputs["s2_b1"], np.float32),
        np.asarray(inputs["s2_w2"], np.float32), np.asarray(inputs["s2_b2"], np.float32))

    gidx = np.empty((P, L * 8), dtype=np.int32)
    for k in range(L):
        for j in range(8):
            gidx[:, k * 8 + j] = perms[k, j * P:(j + 1) * P]
    ones = np.ones((P, 1), dtype=np.float32)

    in_maps = []
    for ci in range(NCORE):
        xs = np.ascontiguousarray(x[ci * BS:(ci + 1) * BS].T)
        cs = np.ascontiguousarray(cond[ci * BS:(ci + 1) * BS].T)
        in_maps.append(dict(x=xs, cond=cs, wts=wts, bias=bias,
                            gidx=gidx, ones=ones))

    nc = get_nc()
    res = run_bass_kernel_spmd(nc, in_maps, core_ids=list(range(NCORE)),
                               trace=trace)

    x_out = np.empty((B, D), dtype=np.float32)
    jac_out = np.empty((B,), dtype=np.float32)
    for ci in range(NCORE):
        x_out[ci * BS:(ci + 1) * BS] = res.results[ci]["out"].T
        jac_out[ci * BS:(ci + 1) * BS] = res.results[ci]["jac"][0]
    return (x_out, jac_out), res


def kernel(**inputs):
    out, _ = _run(inputs, trace=False)
    return out


def kernel_traced(**inputs):
    return _run(inputs, trace=True)


# revision 2
# speedup vs baseline: 1.1649x; 1.1649x over previous
"""GLOW coupling-flow (FrEIA-style) forward pass on 8 TRN2 NeuronCores.

Problem: B=8192, D=1024, C=512, H=512, L=8 coupling layers, each:
    xp = x[:, perm_k];  x1, x2 = xp[:, :512], xp[:, 512:]
    r2 = relu([x2, cond] @ w1 + b1) @ w2 + b2          (subnet 2)
    ls2 = 0.636*atan(r2[:, :512]);  y1 = exp(ls2)*x1 + r2[:, 512:]
    r1 = relu([y1, cond] @ w1 + b1) @ w2 + b2          (subnet 1)
    ls1 = 0.636*atan(r1[:, :512]);  y2 = exp(ls1)*x2 + r1[:, 512:]
    jac += sum(ls1 + ls2, axis=1);  x = [y1, y2]

Strategy:
- Pure data parallel: batch sharded 1024 rows/core, no collectives.
- Activations transposed (features on partitions, batch on free axis):
  weight-stationary matmuls  out^T = W^T @ inp^T  with K on partitions.
- fp16 matmuls (1 cyc/row, FWL weight loads, 2-byte traffic; end-to-end
  rel err ~3e-3 vs fp32 reference, gate is 2e-2).
- Column permutation per layer via DRAM round-trip + one dma_gather per
  (layer, chunk) (1024 rows each), hidden behind compute by splitting the
  batch into NCHUNK=4 pipelined chunks.
- ScalarE stays in the sigmoid_and_others table set for relu/atan; Exp
  (exp_and_others) is batched per chunk-pair to bound table switches.
- relu and the t-half (bias + add) run on DVE; jac = ones-matmul
  partition sums of atan tiles accumulated in PSUM all kernel long.
"""
import sys

sys.path.insert(0, "/opt/trn_rl_repo")
import numpy as np

import concourse.bass as bass
import concourse.bacc as bacc
import concourse.tile as tile
import concourse.mybir as mybir
from concourse.bass_utils import run_bass_kernel_spmd

F32 = mybir.dt.float32
F16 = mybir.dt.float16
I16 = mybir.dt.int16
AF = mybir.ActivationFunctionType
ALU = mybir.AluOpType

B, D, C, H, L = 8192, 1024, 512, 512, 8
S = D // 2
ATAN_SCALE = 0.636
NCORE = 8
BS = B // NCORE          # 1024 batch rows per core
NCHUNK = 4
BC = BS // NCHUNK        # 256 batch columns per chunk
KT_X = D // 128          # 8
KT_S = S // 128          # 4
P = 128

_NC_CACHE = {}


def build():
    nc = bacc.Bacc("TRN2", target_bir_lowering=False)

    x_in = nc.dram_tensor("x", [D, BS], F16, kind="ExternalInput")
    cond_in = nc.dram_tensor("cond", [C, BS], F16, kind="ExternalInput")
    # per (layer, subnet): [128, 8*512 (w1, kt-major) + 4*1024 (w2)]
    wts = nc.dram_tensor("wts", [L, 2, P, 8192], F16, kind="ExternalInput")
    # per (layer, subnet) 12 cols: b1 (4 mt) then b2 (8 mt)
    bias_in = nc.dram_tensor("bias", [P, L * 2 * 12], F32, kind="ExternalInput")
    gidx_in = nc.dram_tensor("gidx", [P, L * 64], I16, kind="ExternalInput")
    ones_in = nc.dram_tensor("ones", [P, 1], F16, kind="ExternalInput")
    out_x = nc.dram_tensor("out", [D, BS], F16, kind="ExternalOutput")
    out_j = nc.dram_tensor("jac", [1, BS], F32, kind="ExternalOutput")
    xb = [nc.dram_tensor(f"xbuf{i}", [D, BS], F16) for i in range(2)]

    def dram_T(t, rows_lo, rows_hi, col_lo, col_hi):
        return t[rows_lo:rows_hi, col_lo:col_hi].rearrange(
            "(kt p) n -> p kt n", p=P)

    with tile.TileContext(nc) as tc:
        with (
            tc.tile_pool(name="const", bufs=1) as cpool,
            tc.tile_pool(name="wt", bufs=2) as wpool,
            tc.tile_pool(name="hp", bufs=3) as hpool,
            tc.tile_pool(name="aep", bufs=3) as aepool,
            tc.tile_pool(name="ab", bufs=6) as abpool,
            tc.tile_pool(name="ps", bufs=6, space="PSUM") as pspool,
            tc.tile_pool(name="psj", bufs=1, space="PSUM") as psjpool,
        ):
            # --- persistent loads ---
            ct = cpool.tile([P, KT_S, BS], F16, tag="cond")
            nc.sync.dma_start(ct[:], dram_T(cond_in, 0, C, 0, BS))
            bsb = cpool.tile([P, L * 2 * 12], F32, tag="bias")
            nc.sync.dma_start(bsb[:], bias_in[:])
            gsb = cpool.tile([P, L * 64], I16, tag="gidx")
            nc.sync.dma_start(gsb[:], gidx_in[:])
            ones_t = cpool.tile([P, 1], F16, tag="ones")
            nc.sync.dma_start(ones_t[:], ones_in[:])
            z0 = cpool.tile([P, 1], F32, tag="z0")
            nc.gpsimd.memset(z0[:], 0.0)

            jac_ps = psjpool.tile([1, BS], F32, tag="jac")
            jac_started = [False] * 2          # per pair (512-col slice)

            for k in range(L):
                src = x_in if k == 0 else xb[(k - 1) % 2]
                dst = out_x if k == L - 1 else xb[k % 2]

                w_t = []
                for s in range(2):
                    w = wpool.tile([P, 8192], F16, tag="w")
                    nc.sync.dma_start(w[:], wts[k, s])
                    w_t.append(w)

                # one dma_gather per chunk: ab[p, j, :] = src[perm[j*128+p], cols]
                ab = []
                for c in range(NCHUNK):
                    t = abpool.tile([P, KT_X, BC], F16, tag="ab")
                    nc.gpsimd.dma_gather(
                        out_ap=t[:], in_ap=src[:, c * BC:(c + 1) * BC],
                        idxs_ap=gsb[:, k * 64:(k + 1) * 64],
                        num_idxs=BS, num_idxs_reg=BS,
                        elem_size=BC, elem_step=BS)
                    ab.append(t)

                # s=0 -> subnet2 (input x2 = ab[4:8], writes y1 into ab[0:4])
                # s=1 -> subnet1 (input y1 = ab[0:4], writes y2 into ab[4:8])
                for s in range(2):
                    bofs = (k * 2 + s) * 12
                    for pair in range(NCHUNK // 2):
                        cpair = (2 * pair, 2 * pair + 1)
                        A = aepool.tile([P, KT_S, 2 * BC], F16, tag="A")
                        E = aepool.tile([P, KT_S, 2 * BC], F16, tag="E")
                        hs = {}
                        for c in cpair:
                            cs, ce = c * BC, (c + 1) * BC
                            off = (c & 1) * BC
                            inp_lo = 4 * (1 - s)   # kt base of subnet input
                            # mm1 (cond k-tiles first, gathered input last)
                            h = hpool.tile([P, KT_S, BC], F16, tag="h")
                            hs[c] = h
                            for mt in range(KT_S):
                                ph = pspool.tile([P, BC], F32, tag="ps")
                                kts = list(range(KT_S, KT_X)) + list(range(KT_S))
                                for i, kt in enumerate(kts):
                                    if kt < KT_S:
                                        rhs = ab[c][:, inp_lo + kt, :]
                                    else:
                                        rhs = ct[:, kt - KT_S, cs:ce]
                                    lo = kt * 512 + mt * P
                                    nc.tensor.matmul(
                                        ph[:], w_t[s][:, lo:lo + P], rhs,
                                        start=(i == 0), stop=(i == KT_X - 1))
                                nc.vector.tensor_scalar(
                                    out=h[:, mt, :], in0=ph[:],
                                    scalar1=bsb[:, bofs + mt:bofs + mt + 1],
                                    scalar2=0.0, op0=ALU.add, op1=ALU.max)
                            # mm2 s-half -> atan
                            for mt in range(KT_S):
                                pr = pspool.tile([P, BC], F32, tag="ps")
                                for kt in range(KT_S):
                                    lo = 4096 + kt * 1024 + mt * P
                                    nc.tensor.matmul(
                                        pr[:], w_t[s][:, lo:lo + P], h[:, kt, :],
                                        start=(kt == 0), stop=(kt == KT_S - 1))
                                nc.scalar.activation(
                                    A[:, mt, off:off + BC], pr[:], AF.Arctan,
                                    bias=bsb[:, bofs + 4 + mt:bofs + 5 + mt],
                                    scale=1.0)
                        # jac partition-sums over the whole pair (N=512)
                        pcs = pair * 2 * BC
                        for mt in range(KT_S):
                            nc.tensor.matmul(
                                jac_ps[:, pcs:pcs + 2 * BC], ones_t[:],
                                A[:, mt, :], start=not jac_started[pair],
                                stop=False, skip_group_check=True)
                            jac_started[pair] = True
                        # exp for both chunks of the pair (one table switch in)
                        nc.scalar.activation(E[:], A[:], AF.Exp,
                                             bias=z0[:, :1], scale=ATAN_SCALE)
                        # mm2 t-half + y update
                        for c in cpair:
                            off = (c & 1) * BC
                            xm_lo = 4 * s       # kt base of mult operand
                            tmp = aepool.tile([P, KT_S, BC], F16, tag="tmp")
                            nc.vector.tensor_mul(
                                tmp[:], E[:, :, off:off + BC],
                                ab[c][:, xm_lo:xm_lo + KT_S, :])
                            for mt in range(KT_S, 2 * KT_S):
                                pr = pspool.tile([P, BC], F32, tag="ps")
                                for kt in range(KT_S):
                                    lo = 4096 + kt * 1024 + mt * P
                                    nc.tensor.matmul(
                                        pr[:], w_t[s][:, lo:lo + P],
                                        hs[c][:, kt, :],
                                        start=(kt == 0), stop=(kt == KT_S - 1))
                                j = mt - KT_S
                                nc.vector.scalar_tensor_tensor(
                                    out=ab[c][:, xm_lo + j, :], in0=pr[:],
                                    scalar=bsb[:, bofs + 4 + mt:bofs + 5 + mt],
                                    in1=tmp[:, j, :],
                                    op0=ALU.add, op1=ALU.add)
                            if s == 1:
                                cs, ce = c * BC, (c + 1) * BC
                                nc.sync.dma_start(
                                    dram_T(dst, 0, D, cs, ce), ab[c][:])

            jac_sb = cpool.tile([1, BS], F32, tag="jacsb")
            nc.scalar.mul(jac_sb[:], jac_ps[:], ATAN_SCALE)
            nc.sync.dma_start(out_j[:], jac_sb[:])

    nc.compile()
    return nc


def get_nc():
    if "nc" not in _NC_CACHE:
        _NC_CACHE["nc"] = build()
    return _NC_CACHE["nc"]


def _pack_weights(s1_w1, s1_b1, s1_w2, s1_b2, s2_w1, s2_b1, s2_w2, s2_b2):
    wts = np.empty((L, 2, P, 8192), dtype=np.float16)
    bias = np.empty((P, L * 2 * 12), dtype=np.float32)
    # s index 0 -> subnet2 (runs first), 1 -> subnet1
    for k in range(L):
        for s, (w1, b1, w2, b2) in enumerate(
            ((s2_w1, s2_b1, s2_w2, s2_b2), (s1_w1, s1_b1, s1_w2, s1_b2))
        ):
            wts[k, s, :, :4096] = (
                w1[k].reshape(8, P, 512).transpose(1, 0, 2).reshape(P, 4096))
            wts[k, s, :, 4096:] = (
                w2[k].reshape(4, P, 1024).transpose(1, 0, 2).reshape(P, 4096))
            bofs = (k * 2 + s) * 12
            bias[:, bofs:bofs + 4] = b1[k].reshape(4, P).T
            bias[:, bofs + 4:bofs + 12] = b2[k].reshape(8, P).T
    return wts, bias


def _run(inputs, trace=False):
    x = np.asarray(inputs["x"], dtype=np.float32)
    cond = np.asarray(inputs["cond"], dtype=np.float32)
    perms = np.asarray(inputs["perms"]).astype(np.int64)
    wts, bias = _pack_weights(
        np.asarray(inputs["s1_w1"], np.float32), np.asarray(inputs["s1_b1"], np.float32),
        np.asarray(inputs["s1_w2"], np.float32), np.asarray(inputs["s1_b2"], np.float32),
        np.asarray(inputs["s2_w1"], np.float32), np.asarray(inputs["s2_b1"], np.float32),
        np.asarray(inputs["s2_w2"], np.float32), np.asarray(inputs["s2_b2"], np.float32))

    # gather index tiles: 16-partition wrap, replicated across the 8 Q7 cores
    gidx = np.empty((P, L * 64), dtype=np.int16)
    for k in range(L):
        blk = np.zeros((16, 64), dtype=np.int16)
        pk = perms[k]
        for i in range(BS):
            blk[i % 16, i // 16] = pk[i]
        gidx[:, k * 64:(k + 1) * 64] = np.tile(blk, (8, 1))
    ones = np.ones((P, 1), dtype=np.float16)

    in_maps = []
    for ci in range(NCORE):
        xs = np.ascontiguousarray(x[ci * BS:(ci + 1) * BS].T).astype(np.float16)
        cs = np.ascontiguousarray(cond[ci * BS:(ci + 1) * BS].T).astype(np.float16)
        in_maps.append(dict(x=xs, cond=cs, wts=wts, bias=bias,
                            gidx=gidx, ones=ones))

    nc = get_nc()
    res = run_bass_kernel_spmd(nc, in_maps, core_ids=list(range(NCORE)),
                               trace=trace)

    x_out = np.empty((B, D), dtype=np.float32)
    jac_out = np.empty((B,), dtype=np.float32)
    for ci in range(NCORE):
        x_out[ci * BS:(ci + 1) * BS] = res.results[ci]["out"].T.astype(np.float32)
        jac_out[ci * BS:(ci + 1) * BS] = res.results[ci]["jac"][0]
    return (x_out, jac_out), res


def kernel(**inputs):
    out, _ = _run(inputs, trace=False)
    return out


def kernel_traced(**inputs):
    return _run(inputs, trace=True)


# revision 7
# speedup vs baseline: 1.4027x; 1.2042x over previous
"""GLOW coupling-flow (FrEIA-style) forward pass on 8 TRN2 NeuronCores.

Problem: B=8192, D=1024, C=512, H=512, L=8 coupling layers, each:
    xp = x[:, perm_k];  x1, x2 = xp[:, :512], xp[:, 512:]
    r2 = relu([x2, cond] @ w1 + b1) @ w2 + b2          (subnet 2)
    ls2 = 0.636*atan(r2[:, :512]);  y1 = exp(ls2)*x1 + r2[:, 512:]
    r1 = relu([y1, cond] @ w1 + b1) @ w2 + b2          (subnet 1)
    ls1 = 0.636*atan(r1[:, :512]);  y2 = exp(ls1)*x2 + r1[:, 512:]
    jac += sum(ls1 + ls2, axis=1);  x = [y1, y2]

Strategy:
- Pure data parallel: batch sharded 1024 rows/core, no collectives.
- Activations transposed (features on partitions, batch on free axis):
  weight-stationary matmuls  out^T = W^T @ inp^T  with K on partitions.
- fp16 matmuls (1 cyc/row, FWL weight loads, 2-byte traffic; end-to-end
  rel err ~3e-3 vs fp32 reference, gate is 2e-2).
- Column permutation per layer via DRAM round-trip + one dma_gather per
  (layer, chunk) (1024 rows each), hidden behind compute by splitting the
  batch into NCHUNK=4 pipelined chunks.
- ScalarE stays in the sigmoid_and_others table set for relu/atan; Exp
  (exp_and_others) is batched per chunk-pair to bound table switches.
- relu and the t-half (bias + add) run on DVE; jac = ones-matmul
  partition sums of atan tiles accumulated in PSUM all kernel long.
"""
import sys

sys.path.insert(0, "/opt/trn_rl_repo")
import numpy as np

import concourse.bass as bass
import concourse.bacc as bacc
import concourse.tile as tile
import concourse.mybir as mybir
from concourse.bass_utils import run_bass_kernel_spmd

F32 = mybir.dt.float32
F16 = mybir.dt.float16
I16 = mybir.dt.int16
AF = mybir.ActivationFunctionType
ALU = mybir.AluOpType

B, D, C, H, L = 8192, 1024, 512, 512, 8
S = D // 2
ATAN_SCALE = 0.636
NCORE = 8
BS = B // NCORE          # 1024 batch rows per core
NCHUNK = 4
BC = BS // NCHUNK        # 256 batch columns per chunk
KT_X = D // 128          # 8
KT_S = S // 128          # 4
P = 128

_NC_CACHE = {}


def build():
    nc = bacc.Bacc("TRN2", target_bir_lowering=False)

    # X layout in DRAM is chunk-major [NCHUNK, D, BC] so per-chunk writes,
    # gathers, and their dependencies are contiguous, non-overlapping ranges.
    x_in = nc.dram_tensor("x", [NCHUNK, D, BC], F16, kind="ExternalInput")
    cond_in = nc.dram_tensor("cond", [C, BS], F16, kind="ExternalInput")
    # per (layer, subnet): [128, 8*512 (w1, kt-major) + 4*1024 (w2)]
    wts = nc.dram_tensor("wts", [L, 2, P, 8192], F16, kind="ExternalInput")
    # per (layer, subnet) 12 cols: b1 (4 mt) then b2 (8 mt)
    bias_in = nc.dram_tensor("bias", [P, L * 2 * 12], F32, kind="ExternalInput")
    gidx_in = nc.dram_tensor("gidx", [P, L * 64], I16, kind="ExternalInput")
    ones_in = nc.dram_tensor("ones", [P, 1], F16, kind="ExternalInput")
    out_x = nc.dram_tensor("out", [NCHUNK, D, BC], F16, kind="ExternalOutput")
    out_j = nc.dram_tensor("jac", [1, BS], F32, kind="ExternalOutput")
    xb = [nc.dram_tensor(f"xbuf{i}", [NCHUNK, D, BC], F16) for i in range(2)]

    def chunk_T(t, c):
        # view chunk c of a [NCHUNK, D, BC] DRAM tensor as [128, kt, BC]
        return t[c].rearrange("(kt p) n -> p kt n", p=P)

    with tile.TileContext(nc) as tc:
        with (
            tc.tile_pool(name="const", bufs=1) as cpool,
            tc.tile_pool(name="wt", bufs=2) as wpool,
            tc.tile_pool(name="hp", bufs=3) as hpool,
            tc.tile_pool(name="aep", bufs=3) as aepool,
            tc.tile_pool(name="ab", bufs=6) as abpool,
            tc.tile_pool(name="ps", bufs=6, space="PSUM") as pspool,
            tc.tile_pool(name="psj", bufs=1, space="PSUM") as psjpool,
        ):
            # --- persistent loads ---
            ct = cpool.tile([P, KT_S, BS], F16, tag="cond")
            nc.sync.dma_start(ct[:], cond_in.rearrange("(kt p) n -> p kt n", p=P))
            bsb = cpool.tile([P, L * 2 * 12], F32, tag="bias")
            nc.sync.dma_start(bsb[:], bias_in[:])
            gsb = cpool.tile([P, L * 64], I16, tag="gidx")
            nc.sync.dma_start(gsb[:], gidx_in[:])
            ones_t = cpool.tile([P, 1], F16, tag="ones")
            nc.sync.dma_start(ones_t[:], ones_in[:])
            z0 = cpool.tile([P, 1], F32, tag="z0")
            nc.gpsimd.memset(z0[:], 0.0)

            jac_ps = psjpool.tile([1, BS], F32, tag="jac")
            jac_started = [False] * 2          # per pair (512-col slice)

            for k in range(L):
                src = x_in if k == 0 else xb[(k - 1) % 2]
                dst = out_x if k == L - 1 else xb[k % 2]

                w_t = []
                for s in range(2):
                    w = wpool.tile([P, 8192], F16, tag="w")
                    nc.sync.dma_start(w[:], wts[k, s])
                    w_t.append(w)

                # one dma_gather per chunk: ab[p, j, :] = src[c, perm[j*128+p], :]
                ab = []
                for c in range(NCHUNK):
                    t = abpool.tile([P, KT_X, BC], F16, tag="ab")
                    nc.gpsimd.dma_gather(
                        out_ap=t[:], in_ap=src[c],
                        idxs_ap=gsb[:, k * 64:(k + 1) * 64],
                        num_idxs=BS, num_idxs_reg=BS,
                        elem_size=BC, elem_step=BC)
                    ab.append(t)

                # s=0 -> subnet2 (input x2 = ab[4:8], writes y1 into ab[0:4])
                # s=1 -> subnet1 (input y1 = ab[0:4], writes y2 into ab[4:8])
                for s in range(2):
                    bofs = (k * 2 + s) * 12
                    for pair in range(NCHUNK // 2):
                        cpair = (2 * pair, 2 * pair + 1)
                        A = aepool.tile([P, KT_S, 2 * BC], F16, tag="A")
                        E = aepool.tile([P, KT_S, 2 * BC], F16, tag="E")
                        # h is shared across the pair so mm2 runs at N=512
                        h = hpool.tile([P, KT_S, 2 * BC], F16, tag="h")
                        for c in cpair:
                            cs, ce = c * BC, (c + 1) * BC
                            off = (c & 1) * BC
                            inp_lo = 4 * (1 - s)   # kt base of subnet input
                            # mm1 (cond k-tiles first, gathered input last)
                            for mt in range(KT_S):
                                ph = pspool.tile([P, BC], F32, tag="ps")
                                kts = list(range(KT_S, KT_X)) + list(range(KT_S))
                                for i, kt in enumerate(kts):
                                    if kt < KT_S:
                                        rhs = ab[c][:, inp_lo + kt, :]
                                    else:
                                        rhs = ct[:, kt - KT_S, cs:ce]
                                    lo = kt * 512 + mt * P
                                    nc.tensor.matmul(
                                        ph[:], w_t[s][:, lo:lo + P], rhs,
                                        start=(i == 0), stop=(i == KT_X - 1))
                                nc.vector.tensor_scalar(
                                    out=h[:, mt, off:off + BC], in0=ph[:],
                                    scalar1=bsb[:, bofs + mt:bofs + mt + 1],
                                    scalar2=0.0, op0=ALU.add, op1=ALU.max)
                        # mm2 s-half at N=512 -> atan -> A
                        for mt in range(KT_S):
                            pr = pspool.tile([P, 2 * BC], F32, tag="ps")
                            for kt in range(KT_S):
                                lo = 4096 + kt * 1024 + mt * P
                                nc.tensor.matmul(
                                    pr[:], w_t[s][:, lo:lo + P], h[:, kt, :],
                                    start=(kt == 0), stop=(kt == KT_S - 1))
                            nc.scalar.activation(
                                A[:, mt, :], pr[:], AF.Arctan,
                                bias=bsb[:, bofs + 4 + mt:bofs + 5 + mt],
                                scale=1.0)
                        # jac partition-sums over the whole pair (N=512)
                        pcs = pair * 2 * BC
                        for mt in range(KT_S):
                            nc.tensor.matmul(
                                jac_ps[:, pcs:pcs + 2 * BC], ones_t[:],
                                A[:, mt, :], start=not jac_started[pair],
                                stop=False, skip_group_check=True)
                            jac_started[pair] = True
                        # exp for both chunks of the pair (one table switch in)
                        nc.scalar.activation(E[:], A[:], AF.Exp,
                                             bias=z0[:, :1], scale=ATAN_SCALE)
                        # pre-multiply E * x for both chunks
                        tmps = {}
                        for c in cpair:
                            off = (c & 1) * BC
                            xm_lo = 4 * s       # kt base of mult operand
                            tmp = aepool.tile([P, KT_S, BC], F16, tag="tmp")
                            nc.vector.tensor_mul(
                                tmp[:], E[:, :, off:off + BC],
                                ab[c][:, xm_lo:xm_lo + KT_S, :])
                            tmps[c] = tmp
                        # mm2 t-half at N=512 + y update per chunk
                        for mt in range(KT_S, 2 * KT_S):
                            pr = pspool.tile([P, 2 * BC], F32, tag="ps")
                            for kt in range(KT_S):
                                lo = 4096 + kt * 1024 + mt * P
                                nc.tensor.matmul(
                                    pr[:], w_t[s][:, lo:lo + P], h[:, kt, :],
                                    start=(kt == 0), stop=(kt == KT_S - 1))
                            j = mt - KT_S
                            for c in cpair:
                                off = (c & 1) * BC
                                xm_lo = 4 * s
                                nc.vector.scalar_tensor_tensor(
                                    out=ab[c][:, xm_lo + j, :],
                                    in0=pr[:, off:off + BC],
                                    scalar=bsb[:, bofs + 4 + mt:bofs + 5 + mt],
                                    in1=tmps[c][:, j, :],
                                    op0=ALU.add, op1=ALU.add)
                        if s == 1:
                            for c in cpair:
                                nc.sync.dma_start(chunk_T(dst, c), ab[c][:])

            jac_sb = cpool.tile([1, BS], F32, tag="jacsb")
            nc.scalar.mul(jac_sb[:], jac_ps[:], ATAN_SCALE)
            nc.sync.dma_start(out_j[:], jac_sb[:])

    nc.compile()
    return nc


def get_nc():
    if "nc" not in _NC_CACHE:
        _NC_CACHE["nc"] = build()
    return _NC_CACHE["nc"]


def _pack_weights(s1_w1, s1_b1, s1_w2, s1_b2, s2_w1, s2_b1, s2_w2, s2_b2):
    wts = np.empty((L, 2, P, 8192), dtype=np.float16)
    bias = np.empty((P, L * 2 * 12), dtype=np.float32)
    # s index 0 -> subnet2 (runs first), 1 -> subnet1
    for k in range(L):
        for s, (w1, b1, w2, b2) in enumerate(
            ((s2_w1, s2_b1, s2_w2, s2_b2), (s1_w1, s1_b1, s1_w2, s1_b2))
        ):
            wts[k, s, :, :4096] = (
                w1[k].reshape(8, P, 512).transpose(1, 0, 2).reshape(P, 4096))
            wts[k, s, :, 4096:] = (
                w2[k].reshape(4, P, 1024).transpose(1, 0, 2).reshape(P, 4096))
            bofs = (k * 2 + s) * 12
            bias[:, bofs:bofs + 4] = b1[k].reshape(4, P).T
            bias[:, bofs + 4:bofs + 12] = b2[k].reshape(8, P).T
    return wts, bias


def _run(inputs, trace=False):
    x = np.asarray(inputs["x"], dtype=np.float32)
    cond = np.asarray(inputs["cond"], dtype=np.float32)
    perms = np.asarray(inputs["perms"]).astype(np.int64)
    wts, bias = _pack_weights(
        np.asarray(inputs["s1_w1"], np.float32), np.asarray(inputs["s1_b1"], np.float32),
        np.asarray(inputs["s1_w2"], np.float32), np.asarray(inputs["s1_b2"], np.float32),
        np.asarray(inputs["s2_w1"], np.float32), np.asarray(inputs["s2_b1"], np.float32),
        np.asarray(inputs["s2_w2"], np.float32), np.asarray(inputs["s2_b2"], np.float32))

    # gather index tiles: 16-partition wrap, replicated across the 8 Q7 cores
    gidx = np.empty((P, L * 64), dtype=np.int16)
    for k in range(L):
        blk = np.zeros((16, 64), dtype=np.int16)
        pk = perms[k]
        for i in range(BS):
            blk[i % 16, i // 16] = pk[i]
        gidx[:, k * 64:(k + 1) * 64] = np.tile(blk, (8, 1))
    ones = np.ones((P, 1), dtype=np.float16)

    in_maps = []
    for ci in range(NCORE):
        xs = x[ci * BS:(ci + 1) * BS].T.astype(np.float16)          # [D, BS]
        xs = np.ascontiguousarray(
            xs.reshape(D, NCHUNK, BC).transpose(1, 0, 2))           # [NCHUNK, D, BC]
        cs = np.ascontiguousarray(cond[ci * BS:(ci + 1) * BS].T).astype(np.float16)
        in_maps.append(dict(x=xs, cond=cs, wts=wts, bias=bias,
                            gidx=gidx, ones=ones))

    nc = get_nc()
    res = run_bass_kernel_spmd(nc, in_maps, core_ids=list(range(NCORE)),
                               trace=trace)

    x_out = np.empty((B, D), dtype=np.float32)
    jac_out = np.empty((B,), dtype=np.float32)
    for ci in range(NCORE):
        oc = res.results[ci]["out"]                                 # [NCHUNK, D, BC]
        x_out[ci * BS:(ci + 1) * BS] = (
            oc.transpose(1, 0, 2).reshape(D, BS).T.astype(np.float32))
        jac_out[ci * BS:(ci + 1) * BS] = res.results[ci]["jac"][0]
    return (x_out, jac_out), res


def kernel(**inputs):
    out, _ = _run(inputs, trace=False)
    return out


def kernel_traced(**inputs):
    return _run(inputs, trace=True)
